# revision 10
# baseline (speedup 1.0000x reference)
"""Contrastive-loss kernel for Trainium2, 8 NeuronCores, data-parallel over batch.

Problem: a, b [16, 1024, 512] f32. Per batch pairwise squared distances
d2[j,k] = ||a_j||^2 + ||b_k||^2 - 2 a_j.b_k; d = sqrt(d2);
loss = [sum_offdiag d2 + sum_offdiag relu(1-d)^2] / (B*N*(N-1)).

Sharding: 2 batches per core. Each core computes partial sums (pos/hinge over
all pairs, plus the diagonal terms to subtract); host combines.

Per-core pipeline:
  - load a,b natural tiles [128,512]
  - row norms a2/b2 via ACT Square+accum; row dots (diag) via DVE stt+accum
  - transpose both to [d, n] layout via PE transpose; PSUM->SBUF copies write
    float32r (rounded), A side scaled by -2 in the ACT copy
  - b2 row -> DRAM -> partition-broadcast DMA -> [128,512] bcast tiles
  - main loop: 4 f32r matmuls accumulate -2ab in PSUM; DVE stt adds a2 (per
    partition) + b2 (bcast) -> d2, accum_out = pos partial; ACT sqrt; ACT
    relu(1-d); DVE stt h*h with accum_out = hinge partial
  - diagonal: d2diag = a2+b2-2*rowdot, hinge likewise, reduced on-chip
"""

import numpy as np
from contextlib import ExitStack

import concourse.bass as bass
import concourse.tile as tile
from concourse import mybir
import bass_rust
from concourse.bass_utils import run_bass_kernel_spmd
from concourse.masks import make_identity

F32 = mybir.dt.float32
F32R = mybir.dt.float32r

B, N, D = 16, 1024, 512
NCORES = 8
BPC = B // NCORES          # batches per core
NT = N // 128              # 8 n-tiles per batch
NC = D // 128              # 4 contraction chunks
NF = N // 512              # 2 free halves
MARGIN = 1.0

ACT = mybir.ActivationFunctionType
ALU = mybir.AluOpType

# out columns: [0:32] pos partials, [32:64] h2 partials, [64] d2diag, [65] h2diag
OUTC = 66


def _split_multiwaits(nc, max_waits=1):
    # this walrus build accepts only one sync-wait per CTRL instruction;
    # split multi-wait instructions into single-wait drains placed before.
    n_new = 0
    for f in nc.m.functions:
        for bb in f.blocks:
            new_list = []
            changed = False
            for inst in bb.instructions:
                si = inst.sync_info
                if si is not None and len(si.on_wait) > max_waits:
                    waits = list(si.on_wait)
                    for w in waits[:-max_waits]:
                        n_new += 1
                        d = mybir.InstDrain(
                            name=f"I-swsplit-{n_new}", ins=[], outs=[])
                        d.engine = inst.engine
                        d.sync_info = bass_rust.SyncInfo(
                            on_wait=[w], on_update=[])
                        new_list.append(d)
                    si.on_wait = waits[-max_waits:]
                    changed = True
                new_list.append(inst)
            if changed:
                bb.instructions = new_list
    return n_new


def build_kernel():
    nc = bass.Bass()
    a_in = nc.declare_dram_parameter("a", [BPC, N, D], F32, isOutput=False)
    b_in = nc.declare_dram_parameter("b", [BPC, N, D], F32, isOutput=False)
    out_d = nc.declare_dram_parameter("out", [128, OUTC], F32, isOutput=True)

    with tile.TileContext(nc) as tc, ExitStack() as ctx:
        singles = ctx.enter_context(tc.tile_pool(name="singles", bufs=1))
        nat = ctx.enter_context(tc.tile_pool(name="nat", bufs=1))
        tpool = ctx.enter_context(tc.tile_pool(name="tp", bufs=1))
        work = ctx.enter_context(tc.tile_pool(name="work", bufs=3))
        scratch = ctx.enter_context(tc.tile_pool(name="scr", bufs=2))
        ps_tp = ctx.enter_context(tc.tile_pool(name="ps_tp", bufs=3, space="PSUM"))
        ps_b2 = ctx.enter_context(tc.tile_pool(name="ps_b2", bufs=1, space="PSUM"))
        ps_mm = ctx.enter_context(tc.tile_pool(name="ps_mm", bufs=3, space="PSUM"))

        ident = singles.tile([128, 128], F32)
        make_identity(nc, ident)

        outt = singles.tile([128, OUTC], F32)

        # ---- load natural tiles
        A = {}
        Bn = {}
        for q in range(BPC):
            for t in range(NT):
                at = nat.tile([128, D], F32, tag=f"A{q}_{t}")
                nc.gpsimd.dma_start(
                    out=at, in_=a_in[q, t * 128:(t + 1) * 128, :])
                A[q, t] = at
                bt = nat.tile([128, D], F32, tag=f"B{q}_{t}")
                nc.gpsimd.dma_start(
                    out=bt, in_=b_in[q, t * 128:(t + 1) * 128, :])
                Bn[q, t] = bt

        # ---- row norms + row dots
        a2 = {}
        b2 = {}
        rowdot = {}
        for q in range(BPC):
            a2c = singles.tile([128, NT], F32, tag=f"a2_{q}")
            b2c = singles.tile([128, NT], F32, tag=f"b2_{q}")
            rdc = singles.tile([128, NT], F32, tag=f"rd_{q}")
            a2[q], b2[q], rowdot[q] = a2c, b2c, rdc
            for t in range(NT):
                sq = scratch.tile([128, D], F32, tag="sq")
                nc.scalar.activation(out=sq, in_=A[q, t], func=ACT.Square,
                                     accum_out=a2c[:, t:t + 1])
                sq2 = scratch.tile([128, D], F32, tag="sq")
                nc.scalar.activation(out=sq2, in_=Bn[q, t], func=ACT.Square,
                                     accum_out=b2c[:, t:t + 1])
                pr = scratch.tile([128, D], F32, tag="pr")
                nc.vector.scalar_tensor_tensor(
                    out=pr, in0=A[q, t], scalar=0.0, in1=Bn[q, t],
                    op0=ALU.bypass, op1=ALU.mult,
                    accum_out=rdc[:, t:t + 1])

        # ---- transposes to [d, n] layout, f32r, A scaled by -2
        AT = {}
        BT = {}
        for q in range(BPC):
            for c in range(NC):
                atr = tpool.tile([128, N], F32R, tag=f"AT{q}_{c}")
                btr = tpool.tile([128, N], F32R, tag=f"BT{q}_{c}")
                AT[q, c], BT[q, c] = atr, btr
                for t in range(NT):
                    pst = ps_tp.tile([128, 128], F32, tag="tp")
                    nc.tensor.transpose(
                        pst, A[q, t][:, c * 128:(c + 1) * 128], ident)
                    nc.scalar.mul(
                        out=atr[:, t * 128:(t + 1) * 128], in_=pst, mul=-2.0)
                    pst2 = ps_tp.tile([128, 128], F32, tag="tp")
                    nc.tensor.transpose(
                        pst2, Bn[q, t][:, c * 128:(c + 1) * 128], ident)
                    nc.vector.tensor_copy(
                        out=btr[:, t * 128:(t + 1) * 128], in_=pst2)

        # ---- b2 broadcast tiles: b2 col -> row (transpose + DRAM flatten),
        # then ones[1,128].T @ b2row -> [128,512] partition-broadcast
        ones1 = singles.tile([1, 128], F32)
        nc.vector.memset(ones1, 1.0)
        B2b = {}
        for q in range(BPC):
            psb = ps_b2.tile([128, 128], F32, tag="tpb")
            nc.tensor.transpose(psb[0:NT, :], b2[q], ident)
            b2t = scratch.tile([NT, 128], F32, tag="b2t")
            nc.scalar.copy(out=b2t, in_=psb[0:NT, :])
            b2row = singles.tile([1, N], F32, tag=f"b2row_{q}")
            nc.gpsimd.dma_start(out=b2row, in_=b2t)
            for f in range(NF):
                psbb = ps_b2.tile([128, 512], F32, tag="bcast")
                nc.tensor.matmul(
                    psbb, ones1, b2row[:, f * 512:(f + 1) * 512],
                    start=True, stop=True)
                bb = singles.tile([128, 512], F32, tag=f"b2b_{q}_{f}")
                nc.scalar.copy(out=bb, in_=psbb)
                B2b[q, f] = bb

        # ---- main pairwise loop
        g = 0
        for q in range(BPC):
            for m in range(NT):
                for f in range(NF):
                    psd = ps_mm.tile([128, 512], F32, tag="mm")
                    for c in range(NC):
                        nc.tensor.matmul(
                            psd,
                            AT[q, c][:, m * 128:(m + 1) * 128],
                            BT[q, c][:, f * 512:(f + 1) * 512],
                            start=(c == 0), stop=(c == NC - 1))
                    d2sb = work.tile([128, 512], F32, tag="d2")
                    nc.vector.scalar_tensor_tensor(
                        out=d2sb, in0=psd, scalar=a2[q][:, m:m + 1],
                        in1=B2b[q, f], op0=ALU.add, op1=ALU.add,
                        accum_out=outt[:, g:g + 1])
                    dd = work.tile([128, 512], F32, tag="d")
                    nc.scalar.activation(out=dd, in_=d2sb, func=ACT.Sqrt)
                    hh = work.tile([128, 512], F32, tag="h")
                    nc.scalar.activation(out=hh, in_=dd, func=ACT.Relu,
                                         scale=-1.0, bias=float(MARGIN))
                    h2 = work.tile([128, 512], F32, tag="h2")
                    nc.vector.scalar_tensor_tensor(
                        out=h2, in0=hh, scalar=0.0, in1=hh,
                        op0=ALU.bypass, op1=ALU.mult,
                        accum_out=outt[:, 32 + g:32 + g + 1])
                    g += 1

        # ---- diagonal terms
        dall = singles.tile([128, BPC * NT], F32, tag="dall")
        for q in range(BPC):
            apb = scratch.tile([128, NT], F32, tag="apb")
            nc.vector.tensor_tensor(
                out=apb, in0=a2[q], in1=b2[q], op=ALU.add)
            nc.vector.scalar_tensor_tensor(
                out=dall[:, q * NT:(q + 1) * NT], in0=rowdot[q],
                scalar=-2.0, in1=apb, op0=ALU.mult, op1=ALU.add)
        # d2diag total = sum over all BPC*NT cols
        nc.vector.tensor_reduce(
            out=outt[:, 64:65], in_=dall, axis=mybir.AxisListType.X,
            op=ALU.add)
        ddiag = scratch.tile([128, BPC * NT], F32, tag="ddiag")
        nc.scalar.activation(out=ddiag, in_=dall, func=ACT.Sqrt)
        hdiag = scratch.tile([128, BPC * NT], F32, tag="hdiag")
        nc.scalar.activation(out=hdiag, in_=ddiag, func=ACT.Relu,
                             scale=-1.0, bias=float(MARGIN))
        h2diag = scratch.tile([128, BPC * NT], F32, tag="h2diag")
        nc.vector.scalar_tensor_tensor(
            out=h2diag, in0=hdiag, scalar=0.0, in1=hdiag,
            op0=ALU.bypass, op1=ALU.mult,
            accum_out=outt[:, 65:66])

        nc.gpsimd.dma_start(out=out_d[:, :], in_=outt)

    nc.finalize()
    _split_multiwaits(nc)
    return nc


_NC_CACHE = None


def _get_nc():
    global _NC_CACHE
    if _NC_CACHE is None:
        _NC_CACHE = build_kernel()
    return _NC_CACHE


def kernel(a: np.ndarray, b: np.ndarray, _results_out=None) -> np.ndarray:
    a = np.ascontiguousarray(a, dtype=np.float32)
    b = np.ascontiguousarray(b, dtype=np.float32)
    assert a.shape == (B, N, D) and b.shape == (B, N, D)
    nc = _get_nc()
    in_maps = [
        {"a": a[i * BPC:(i + 1) * BPC], "b": b[i * BPC:(i + 1) * BPC]}
        for i in range(NCORES)
    ]
    res = run_bass_kernel_spmd(nc, in_maps, core_ids=list(range(NCORES)))
    if _results_out is not None:
        _results_out.append(res)
    pos = 0.0
    h2s = 0.0
    d2diag = 0.0
    h2diag = 0.0
    for i in range(NCORES):
        o = res.results[i]["out"].astype(np.float64)
        pos += o[:, 0:32].sum()
        h2s += o[:, 32:64].sum()
        d2diag += o[:, 64].sum()
        h2diag += o[:, 65].sum()
    n_neg = float(B) * N * (N - 1)
    loss = (pos - d2diag + h2s - h2diag) / n_neg
    return np.float32(loss)
